# revision 8
# baseline (speedup 1.0000x reference)
"""CapsuleNet GNN message-passing kernel for 8 Trainium2 NeuronCores.

Reference model: h = relu(x@pca_w+b); 3 routing layers (each: per-capsule
l2norm -> neighbor gather -> 6 dynamic-routing iterations); mlp head with
log_softmax.  Returns (log_probs, h).

Sharding: nodes are padded 30000->30720 and split 3840/core across 8 cores.
Each layer builds a per-core [3840,256] normalized feature table shard,
AllGathers the full [30720,256] table, then dma_gather's each node's 16
neighbor rows and runs all routing iterations locally on-chip.
"""

import os
import numpy as np
from contextlib import ExitStack

N = 30000
M = 16
NFEAT = 1024
NCLASS = 16
K, DD = 8, 32
D = K * DD  # 256
NLAYER = 3
NCORES = 8

_COMPILE_CACHE = {}


def _bc(ap, pos, count):
    """Insert a broadcast (step-0) dim of size `count` at free-dim position
    `pos` (0 = right after the partition dim)."""
    import concourse.bass as bass

    dims = [list(x) for x in ap.ap]
    dims.insert(1 + pos, [0, count])
    return bass.AP(ap.tensor, ap.offset, dims)


def build_program(nsh, nrows, routit, n_cores, enable_asserts=False):
    """Build the SPMD Bass program. nsh = nodes per shard (multiple of 128),
    nrows = global padded node count (= nsh * n_cores)."""
    import concourse.bacc as bacc
    import concourse.tile as tile
    from concourse import mybir

    f32 = mybir.dt.float32
    f32r = mybir.dt.float32r
    i16 = mybir.dt.int16
    AF = mybir.ActivationFunctionType
    OP = mybir.AluOpType
    AX = mybir.AxisListType

    nt = nsh // 128
    assert nsh % 128 == 0

    nc = bacc.Bacc(
        "TRN2",
        target_bir_lowering=False,
        debug=False,
        enable_asserts=enable_asserts,
        num_devices=n_cores,
    )

    # ---- I/O ----
    xT = nc.dram_tensor("xT", [NFEAT, nsh], f32r, kind="ExternalInput")
    idx = nc.dram_tensor("idx", [nsh, M], mybir.dt.int32, kind="ExternalInput")
    pca_w = nc.dram_tensor("pca_w", [NFEAT, D], f32r, kind="ExternalInput")
    pca_b_r = nc.dram_tensor("pca_b_r", [128, D], f32, kind="ExternalInput")
    wbd = nc.dram_tensor("wbd", [D, D], f32r, kind="ExternalInput")
    fc_b_r = nc.dram_tensor("fc_b_r", [128, D], f32, kind="ExternalInput")
    mlp_wT = nc.dram_tensor("mlp_wT", [D, NCLASS], f32r, kind="ExternalInput")
    mlp_b_r = nc.dram_tensor("mlp_b_r", [128, NCLASS], f32, kind="ExternalInput")
    consts = nc.dram_tensor("consts", [128, 8], f32, kind="ExternalInput")
    ident = nc.dram_tensor("ident", [128, 128], f32, kind="ExternalInput")

    out_lp = nc.dram_tensor("out_lp", [nsh, NCLASS], f32, kind="ExternalOutput")
    out_h = nc.dram_tensor("out_h", [nsh, D], f32, kind="ExternalOutput")

    local_t = [nc.dram_tensor(f"local{L}", [nsh, D], f32) for L in range(NLAYER)]
    table_t = [
        nc.dram_tensor(f"table{L}", [nrows, D], f32, addr_space="Shared")
        for L in range(NLAYER)
    ]

    with tile.TileContext(nc) as tc, ExitStack() as ctx:
        cpool = ctx.enter_context(tc.tile_pool(name="cpool", bufs=1))
        wpool = ctx.enter_context(tc.tile_pool(name="wpool", bufs=2))
        zpool = ctx.enter_context(tc.tile_pool(name="zpool", bufs=2))
        tpool = ctx.enter_context(tc.tile_pool(name="tpool", bufs=2))
        spool = ctx.enter_context(tc.tile_pool(name="spool", bufs=3))
        pspool = ctx.enter_context(tc.tile_pool(name="pspool", bufs=2, space="PSUM"))

        # ---- resident constants ----
        pcaw_sb = cpool.tile([128, 8, D], f32r, name="pcaw_sb")
        for kt in range(8):
            nc.sync.dma_start(pcaw_sb[:, kt, :], pca_w[kt * 128 : (kt + 1) * 128, :])
        wbd_sb = cpool.tile([128, 2, D], f32r, name="wbd_sb")
        for h in range(2):
            nc.sync.dma_start(wbd_sb[:, h, :], wbd[h * 128 : (h + 1) * 128, :])
        mlpw_sb = cpool.tile([128, 2, NCLASS], f32r, name="mlpw_sb")
        for h in range(2):
            nc.sync.dma_start(mlpw_sb[:, h, :], mlp_wT[h * 128 : (h + 1) * 128, :])
        pcab_sb = cpool.tile([128, D], f32, name="pcab_sb")
        nc.sync.dma_start(pcab_sb[:], pca_b_r[:])
        fcb_sb = cpool.tile([128, D], f32, name="fcb_sb")
        nc.sync.dma_start(fcb_sb[:], fc_b_r[:])
        mlpb_sb = cpool.tile([128, NCLASS], f32, name="mlpb_sb")
        nc.sync.dma_start(mlpb_sb[:], mlp_b_r[:])
        cst_sb = cpool.tile([128, 8], f32, name="cst_sb")
        nc.sync.dma_start(cst_sb[:], consts[:])
        id_sb = cpool.tile([128, 128], f32, name="id_sb")
        nc.sync.dma_start(id_sb[:], ident[:])
        idx_sb = cpool.tile([128, nt, M], mybir.dt.int32, name="idx_sb")
        for t in range(nt):
            nc.sync.dma_start(idx_sb[:, t, :], idx[t * 128 : (t + 1) * 128, :])

        param_ap = cst_sb[:, 0:1]
        q_ap = cst_sb[:, 1:2]
        w0_ap = cst_sb[:, 2:3]

        def l2norm(src, dst_pool_tag, outdt=f32):
            """per-capsule l2 normalize src [128,256] -> new tile"""
            sq = spool.tile([128, D], f32, name=f"sq_{dst_pool_tag}", tag="sq")
            nc.vector.tensor_mul(sq[:], src[:], src[:])
            ss = spool.tile([128, K], f32, name=f"ss_{dst_pool_tag}", tag="ss")
            nc.vector.tensor_reduce(
                ss[:], sq[:].rearrange("p (k d) -> p k d", k=K), axis=AX.X, op=OP.add
            )
            nrm = spool.tile([128, K], f32, name=f"nrm_{dst_pool_tag}", tag="nrm")
            nc.scalar.sqrt(nrm[:], ss[:])
            nc.vector.tensor_scalar_max(nrm[:], nrm[:], 1e-12)
            rn = spool.tile([128, K], f32, name=f"rn_{dst_pool_tag}", tag="rn")
            nc.vector.reciprocal(rn[:], nrm[:])
            out = wpool.tile([128, D], outdt, name=f"xn_{dst_pool_tag}", tag=f"xn_{dst_pool_tag}")
            nc.vector.tensor_tensor(
                out[:].rearrange("p (k d) -> p k d", k=K),
                src[:].rearrange("p (k d) -> p k d", k=K),
                _bc(rn[:], 1, DD),
                op=OP.mult,
            )
            return out

        def transpose_256(src):
            """src [128,256] -> two sbuf tiles [128,128] holding src.T halves"""
            halves = []
            for h in range(2):
                pst = pspool.tile([128, 128], f32, name=f"pst{h}", tag="pst")
                nc.tensor.transpose(pst[:], src[:, h * 128 : (h + 1) * 128], id_sb[:])
                sb = wpool.tile([128, 128], f32r, name=f"srcT{h}", tag=f"srcT{h}")
                nc.scalar.copy(sb[:], pst[:])
                halves.append(sb)
            return halves

        # =========== stage 1+2: pca + fc + norm -> local0 ===========
        for t in range(nt):
            ps = pspool.tile([128, D], f32, name="ps_pca", tag="ps_pca")
            for kt in range(8):
                xt = wpool.tile([128, 128], f32r, name="xt", tag="xt", bufs=3)
                nc.sync.dma_start(
                    xt[:], xT[kt * 128 : (kt + 1) * 128, t * 128 : (t + 1) * 128]
                )
                nc.tensor.matmul(
                    ps[:],
                    lhsT=xt[:],
                    rhs=pcaw_sb[:, kt, :],
                    start=(kt == 0),
                    stop=(kt == 7),
                )
            hb = wpool.tile([128, D], f32, name="hb", tag="hb")
            nc.vector.tensor_add(hb[:], ps[:], pcab_sb[:])
            nc.scalar.activation(hb[:], hb[:], AF.Relu)
            xd = l2norm(hb, "xd")
            xdT = transpose_256(xd)
            psf = pspool.tile([128, D], f32, name="psf", tag="psf")
            for h in range(2):
                nc.tensor.matmul(
                    psf[:],
                    lhsT=xdT[h][:],
                    rhs=wbd_sb[:, h, :],
                    start=(h == 0),
                    stop=(h == 1),
                )
            xf = wpool.tile([128, D], f32, name="xf", tag="xf")
            nc.vector.tensor_add(xf[:], psf[:], fcb_sb[:])
            nc.scalar.activation(xf[:], xf[:], AF.Relu)
            xn = l2norm(xf, "l0")
            nc.sync.dma_start(local_t[0][t * 128 : (t + 1) * 128, :], xn[:])

        # =========== layers ===========
        for L in range(NLAYER):
            nc.gpsimd.collective_compute(
                "AllGather",
                OP.bypass,
                replica_groups=[list(range(n_cores))],
                ins=[local_t[L][:, :]],
                outs=[table_t[L][:, :]],
            )
            for t in range(nt):
                z = zpool.tile([128, M, D], f32, name="z", tag="z")
                import concourse.bass as bass_mod
                for m in range(M):
                    nc.gpsimd.indirect_dma_start(
                        out=z[:, m, :],
                        out_offset=None,
                        in_=table_t[L][:, :],
                        in_offset=bass_mod.IndirectOffsetOnAxis(
                            ap=idx_sb[:, t, m : m + 1], axis=0
                        ),
                    )
                u0 = wpool.tile([128, D], f32, name="u0", tag="u0")
                nc.sync.dma_start(u0[:], local_t[L][t * 128 : (t + 1) * 128, :])

                zv = z[:].rearrange("p m (k d) -> p m k d", k=K)

                # ---- iteration 0: p = 0 -> w = w0 everywhere ----
                agg = wpool.tile([128, D], f32, name="agg", tag="agg")
                nc.vector.tensor_reduce(
                    agg[:].rearrange("p (k d) -> p k d", k=K),
                    zv.rearrange("p m k d -> p k d m"),
                    axis=AX.X,
                    op=OP.add,
                )
                ub = wpool.tile([128, D], f32, name="ub", tag="ub")
                nc.vector.scalar_tensor_tensor(
                    ub[:], agg[:], w0_ap, u0[:], op0=OP.mult, op1=OP.add
                )
                un = l2norm(ub, "un")

                for it in range(1, routit):
                    tmp = tpool.tile([128, M, K, DD], f32, name="tmp", tag="tmp")
                    nc.vector.tensor_tensor(
                        tmp[:],
                        zv,
                        _bc(un[:].rearrange("p (k d) -> p k d", k=K), 0, M),
                        op=OP.mult,
                    )
                    p = spool.tile([128, M, K], f32, name="p", tag="p")
                    nc.vector.tensor_reduce(p[:], tmp[:], axis=AX.X, op=OP.add)
                    e = spool.tile([128, M, K], f32, name="e", tag="e")
                    nc.scalar.activation(e[:], p[:], AF.Exp)
                    s1 = spool.tile([128, K], f32, name="s1", tag="s1")
                    nc.vector.tensor_reduce(
                        s1[:], e[:].rearrange("p m k -> p k m"), axis=AX.X, op=OP.add
                    )
                    s2 = spool.tile([128, M], f32, name="s2", tag="s2")
                    nc.vector.tensor_reduce(s2[:], e[:], axis=AX.X, op=OP.add)
                    r1 = spool.tile([128, K], f32, name="r1", tag="r1")
                    nc.vector.reciprocal(r1[:], s1[:])
                    r2 = spool.tile([128, M], f32, name="r2", tag="r2")
                    nc.vector.reciprocal(r2[:], s2[:])
                    nc.vector.tensor_scalar_mul(r2[:], r2[:], q_ap)
                    wf = spool.tile([128, M, K], f32, name="wf", tag="wf")
                    nc.vector.scalar_tensor_tensor(
                        wf[:],
                        _bc(r1[:], 0, M),
                        param_ap,
                        _bc(r2[:], 1, K),
                        op0=OP.mult,
                        op1=OP.add,
                    )
                    w = spool.tile([128, M, K], f32, name="w", tag="w")
                    nc.vector.tensor_mul(w[:], e[:], wf[:])
                    tmp2 = tpool.tile([128, M, K, DD], f32, name="tmp2", tag="tmp2", bufs=1)
                    nc.vector.tensor_tensor(tmp2[:], zv, _bc(w[:], 2, DD), op=OP.mult)
                    agg2 = wpool.tile([128, D], f32, name="agg2", tag="agg2")
                    nc.vector.tensor_reduce(
                        agg2[:].rearrange("p (k d) -> p k d", k=K),
                        tmp2[:].rearrange("p m k d -> p k d m"),
                        axis=AX.X,
                        op=OP.add,
                    )
                    ub_new = wpool.tile([128, D], f32, name="ub", tag="ub")
                    nc.vector.tensor_add(ub_new[:], agg2[:], ub[:])
                    ub = ub_new
                    if it < routit - 1:
                        un = l2norm(ub, "un")

                # ---- layer epilogue ----
                if L < NLAYER - 1:
                    hn = wpool.tile([128, D], f32, name="hn", tag="hn")
                    nc.scalar.activation(hn[:], ub[:], AF.Relu)
                    xnext = l2norm(hn, f"l{L + 1}")
                    nc.sync.dma_start(
                        local_t[L + 1][t * 128 : (t + 1) * 128, :], xnext[:]
                    )
                else:
                    nc.sync.dma_start(out_h[t * 128 : (t + 1) * 128, :], ub[:])
                    uT = transpose_256(ub)
                    pl = pspool.tile([128, NCLASS], f32, name="pl", tag="pl")
                    for h in range(2):
                        nc.tensor.matmul(
                            pl[:],
                            lhsT=uT[h][:],
                            rhs=mlpw_sb[:, h, :],
                            start=(h == 0),
                            stop=(h == 1),
                        )
                    lg = spool.tile([128, NCLASS], f32, name="lg", tag="lg")
                    nc.vector.tensor_add(lg[:], pl[:], mlpb_sb[:])
                    lmax = spool.tile([128, 1], f32, name="lmax", tag="lmax")
                    nc.vector.tensor_reduce(lmax[:], lg[:], axis=AX.X, op=OP.max)
                    nc.vector.tensor_scalar_sub(lg[:], lg[:], lmax[:])
                    le = spool.tile([128, NCLASS], f32, name="le", tag="le")
                    nc.scalar.activation(le[:], lg[:], AF.Exp)
                    lsum = spool.tile([128, 1], f32, name="lsum", tag="lsum")
                    nc.vector.tensor_reduce(lsum[:], le[:], axis=AX.X, op=OP.add)
                    lln = spool.tile([128, 1], f32, name="lln", tag="lln")
                    nc.scalar.activation(lln[:], lsum[:], AF.Ln)
                    nc.vector.tensor_scalar_sub(lg[:], lg[:], lln[:])
                    nc.sync.dma_start(out_lp[t * 128 : (t + 1) * 128, :], lg[:])

    nc.compile()
    return nc


def prep_inputs(
    x, neighbors, pca_w, pca_b, raw_param, fc_w, fc_b, mlp_w, mlp_b, n_cores, nsh
):
    """Host-side sharding/layout prep. Returns list of per-core input dicts."""
    n = x.shape[0]
    npad = nsh * n_cores
    nt = nsh // 128

    xp = np.zeros((npad, NFEAT), np.float32)
    xp[:n] = np.asarray(x, np.float32)
    nbr = np.zeros((npad, M), np.int64)
    nbr[:n] = np.asarray(neighbors)

    param = float(1.0 / (1.0 + np.exp(-np.asarray(raw_param, np.float64)[0])))
    q = 1.0 - param
    w0 = param / M + q / K

    wbd = np.zeros((D, D), np.float32)
    fw = np.asarray(fc_w, np.float32)
    for k in range(K):
        wbd[k * DD : (k + 1) * DD, k * DD : (k + 1) * DD] = fw[k].T

    consts = np.zeros((128, 8), np.float32)
    consts[:, 0] = param
    consts[:, 1] = q
    consts[:, 2] = w0

    common = {
        "pca_w": np.asarray(pca_w, np.float32),
        "pca_b_r": np.broadcast_to(
            np.asarray(pca_b, np.float32)[None, :], (128, D)
        ).copy(),
        "wbd": wbd,
        "fc_b_r": np.broadcast_to(
            np.asarray(fc_b, np.float32).reshape(1, D), (128, D)
        ).copy(),
        "mlp_wT": np.asarray(mlp_w, np.float32).T.copy(),
        "mlp_b_r": np.broadcast_to(
            np.asarray(mlp_b, np.float32)[None, :], (128, NCLASS)
        ).copy(),
        "consts": consts,
        "ident": np.eye(128, dtype=np.float32),
    }

    in_maps = []
    for c in range(n_cores):
        lo = c * nsh
        x_c = xp[lo : lo + nsh]
        idx_c = nbr[lo : lo + nsh].astype(np.int32)
        m = dict(common)
        m["xT"] = np.ascontiguousarray(x_c.T)
        m["idx"] = idx_c
        in_maps.append(m)
    return in_maps


def run_compiled(nc, in_maps, n_cores, trace=False):
    from concourse.bass_utils import run_bass_kernel_spmd

    return run_bass_kernel_spmd(nc, in_maps, list(range(n_cores)), trace=trace)


def kernel(x, neighbors, pca_w, pca_b, raw_param, fc_w, fc_b, mlp_w, mlp_b, routit):
    routit = int(routit)
    n = x.shape[0]
    assert n == N, f"kernel hardcoded for N={N}, got {n}"
    nsh = 3840
    key = (nsh, routit)
    if key not in _COMPILE_CACHE:
        _COMPILE_CACHE[key] = build_program(nsh, nsh * NCORES, routit, NCORES)
    nc = _COMPILE_CACHE[key]

    in_maps = prep_inputs(
        x, neighbors, pca_w, pca_b, raw_param, fc_w, fc_b, mlp_w, mlp_b, NCORES, nsh
    )
    res = run_compiled(nc, in_maps, NCORES, trace=False)
    lp = np.concatenate([res.results[c]["out_lp"] for c in range(NCORES)], axis=0)[:n]
    h = np.concatenate([res.results[c]["out_h"] for c in range(NCORES)], axis=0)[:n]
    return lp, h
